# revision 1
# baseline (speedup 1.0000x reference)
"""Single-head attention (B=4, T=4096, C=1024, H=64) on 8 trn2 NeuronCores.

Sharding: 8 shards = (batch b, query-half h).  Each core receives x[b]
pre-transposed to xT [C=1024, T=4096]; for h==1 the T columns are rotated by
2048 so that "this core's" 2048 queries are always columns 0:2048 (softmax is
permutation-invariant over keys, so rotating the key order changes nothing).
This keeps the SPMD program identical on every core with no rank logic.

Per-core kernel (flash-attention style; the [T,T] score matrix never touches
DRAM):
  phase 1: stream xT in [128,512] tiles; PE computes KVT = [Wk|Wv]^T x^T
           ([128,4096], rows 0:64 = K^T, 64:128 = V^T) and Q^T [64,2048]
           (f32r matmuls, contraction over C in 8 chunks of 128);
           V^T tiles are PE-transposed back to V [s,64] and a ones column is
           appended (-> softmax denominator comes out of the attn@V matmul).
  phase 2: for each 512-wide query chunk: for each 128-key tile,
           PE: scoresT[s=128, t=512] = K_tile^T{64,128}.T @ Q^T{64,512}
           ACT: exp(0.125 * scoresT) -> SBUF   (scores are O(3), no max-sub
           needed for a numerically safe softmax)
           PE: outT[65,512] += V_aug[s,65].T @ exp  (accumulate over 32 tiles)
           then PE-transpose outT back to [t,65], multiply rows by the
           reciprocal of column 64 (the exp-sum), DMA out.
"""

import os
import sys

for _p in ("/opt/trn_rl_repo", "/root/.axon_site/_ro/trn_rl_repo"):
    if os.path.isdir(_p) and _p not in sys.path:
        sys.path.append(_p)

import numpy as np

import concourse.bacc as bacc
import concourse.mybir as mybir
import concourse.tile as tile
from concourse.bass_utils import run_bass_kernel_spmd
from concourse.masks import make_identity

B = 4
T = 4096
C = 1024
H = 64
TQ = T // 2  # queries per core
N_CORES = 8

F32 = mybir.dt.float32
F32R = mybir.dt.float32r

NC_CH = C // 128  # 8 contraction chunks
NSB = T // 512  # 8 key/source blocks of 512
NST = T // 128  # 32 key tiles of 128
NTC = TQ // 512  # 4 query chunks of 512


def _build_module():
    nc = bacc.Bacc("TRN2", target_bir_lowering=False, debug=False, num_devices=N_CORES)

    xT = nc.dram_tensor("xT", [NSB, NC_CH, 128, 512], F32, kind="ExternalInput").ap()
    wkpad = nc.dram_tensor("wkpad", [NC_CH, 128, 2 * H], F32, kind="ExternalInput").ap()
    wqv = nc.dram_tensor("wqv", [NC_CH, 128, 2 * H], F32, kind="ExternalInput").ap()
    out = nc.dram_tensor("out", [TQ, H], F32, kind="ExternalOutput").ap()

    EXP = mybir.ActivationFunctionType.Exp

    with tile.TileContext(nc) as tc:
        with (
            tc.tile_pool(name="const", bufs=1) as const_pool,
            tc.tile_pool(name="xt", bufs=32) as xt_pool,
            tc.tile_pool(name="big", bufs=1) as big_pool,
            tc.tile_pool(name="exp", bufs=6) as exp_pool,
            tc.tile_pool(name="outts", bufs=2) as outts_pool,
            tc.tile_pool(name="small", bufs=4) as small_pool,
            tc.tile_pool(name="p1", bufs=2, space="PSUM") as psum_p1,
            tc.tile_pool(name="psc", bufs=2, space="PSUM") as psum_sc,
            tc.tile_pool(name="pacc", bufs=2, space="PSUM") as psum_acc,
        ):
            # ---- constants ----
            wkpad_sb = const_pool.tile([128, NC_CH, 2 * H], F32R, tag="wkpad")
            wqv_sb = const_pool.tile([128, NC_CH, 2 * H], F32R, tag="wqv")
            ident_f32 = const_pool.tile([128, 128], F32, tag="ident_f32")
            ones_f32 = const_pool.tile([128, NST, 1], F32, tag="ones")
            for c in range(NC_CH):
                nc.sync.dma_start(wkpad_sb[:, c, :], wkpad[c].bitcast(F32R))
                nc.sync.dma_start(wqv_sb[:, c, :], wqv[c].bitcast(F32R))
            make_identity(nc, ident_f32[:])
            nc.gpsimd.memset(ones_f32[:], 1.0)

            # ---- persistent activations ----
            kt_sb = big_pool.tile([128, T], F32R, tag="kt")  # K^T, rows 64: = 0
            qv_sb = big_pool.tile([128, TQ], F32R, tag="qv")  # Q^T | V^T(head)
            vt_f32 = big_pool.tile([128, T], F32, tag="vtf32")  # V^T in rows 64:
            va = big_pool.tile([128, NST, 66], F32R, tag="va")  # V_aug per s-tile
            nc.vector.tensor_copy(va[:, :, 64:65], ones_f32[:])

            # ---- phase 1: projections (emitted in two halves, with phase-2
            # score work interleaved so the scheduler overlaps it with the
            # second half of the x DMA stream) ----
            dma_engines = (nc.sync, nc.gpsimd, nc.scalar)

            def emit_proj_block(sb):
                    # contiguous [128,512] tiles; triggers rotate across three
                    # sequencers (a dma_start costs ~650 ns serially on its
                    # issuing sequencer)
                    xts = []
                    for c in range(NC_CH):
                        xt = xt_pool.tile([128, 512], F32R, tag="xt")
                        if sb < 2:
                            # halve the first blocks so all 16 queues work on
                            # them at once -> earliest possible first matmul
                            dma_engines[c % 3].dma_start(
                                xt[:, 0:256], xT[sb, c, :, 0:256].bitcast(F32R)
                            )
                            dma_engines[(c + 1) % 3].dma_start(
                                xt[:, 256:512], xT[sb, c, :, 256:512].bitcast(F32R)
                            )
                        else:
                            dma_engines[c % 3].dma_start(xt[:], xT[sb, c].bitcast(F32R))
                        xts.append(xt)
                    kt_ps = psum_p1.tile([128, 512], F32, tag="p1")
                    for c in range(NC_CH):
                        nc.tensor.matmul(
                            kt_ps[:],
                            wkpad_sb[:, c, :],
                            xts[c][:],
                            start=(c == 0),
                            stop=(c == NC_CH - 1),
                        )
                    nc.vector.tensor_copy(kt_sb[:, sb * 512 : (sb + 1) * 512], kt_ps[:])
                    qv_ps = psum_p1.tile([128, 512], F32, tag="p1")
                    for c in range(NC_CH):
                        nc.tensor.matmul(
                            qv_ps[:],
                            wqv_sb[:, c, :],
                            xts[c][:],
                            start=(c == 0),
                            stop=(c == NC_CH - 1),
                        )
                    nc.vector.tensor_copy(
                        vt_f32[64:128, sb * 512 : (sb + 1) * 512], qv_ps[64:128, :]
                    )
                    if sb < NTC:  # query half: keep Q^T (rows 64: are V^T, benign)
                        nc.vector.tensor_copy(qv_sb[:, sb * 512 : (sb + 1) * 512], qv_ps[:])
                    for j in range(4):  # V tiles of this block
                        st = sb * 4 + j
                        vt_ps = psum_p1.tile([128, 64], F32, tag="p1")
                        nc.tensor.transpose(
                            vt_ps[:],
                            vt_f32[64:128, st * 128 : (st + 1) * 128],
                            ident_f32[64:128, 64:128],
                        )
                        nc.vector.tensor_copy(va[:, st, 0:64], vt_ps[:])


            # ---- phase 2: attention, two query chunks (1024 queries) at a time ----
            outt_tiles = {}

            def emit_attn(tcp, st_lo, st_hi):
                tc0 = 2 * tcp
                if tcp not in outt_tiles:
                    oa = psum_acc.tile([65, 512], F32, tag="acc", name=f"outt_a{tcp}")
                    ob = psum_acc.tile([65, 512], F32, tag="acc", name=f"outt_b{tcp}")
                    outt_tiles[tcp] = (oa, ob)
                outt_a, outt_b = outt_tiles[tcp]
                for st in range(st_lo, st_hi):
                    kt_slice = kt_sb[:, st * 128 : (st + 1) * 128]
                    sc_ps = psum_sc.tile([128, 1024], F32, tag="sc")
                    for i in range(2):
                        nc.tensor.matmul(
                            sc_ps[:, i * 512 : (i + 1) * 512],
                            kt_slice,
                            qv_sb[:, (tc0 + i) * 512 : (tc0 + i + 1) * 512],
                            start=True,
                            stop=True,
                        )
                    ex = exp_pool.tile([128, 1024], F32R, tag="exp")
                    nc.scalar.activation(ex[:], sc_ps[:], EXP, scale=0.125)
                    for i, outt_ps in enumerate((outt_a, outt_b)):
                        nc.tensor.matmul(
                            outt_ps[:],
                            va[:, st, 0:65],
                            ex[:, i * 512 : (i + 1) * 512],
                            start=(st == 0),
                            stop=(st == NST - 1),
                        )

            def emit_epilogue(tcp):
                tc0 = 2 * tcp
                for i, outt_ps in enumerate(outt_tiles[tcp]):
                    tci = tc0 + i
                    outt_sb = outts_pool.tile([65, 512], F32, tag="outts")
                    nc.vector.tensor_copy(outt_sb[:], outt_ps[:])
                    for k in range(4):
                        o_ps = psum_p1.tile([128, 65], F32, tag="p1")
                        nc.tensor.transpose(
                            o_ps[:], outt_sb[:, k * 128 : (k + 1) * 128], ident_f32[0:65, 0:65]
                        )
                        rc = small_pool.tile([128, 1], F32, tag="rc")
                        nc.vector.reciprocal(rc[:], o_ps[:, 64:65])
                        o_sb = small_pool.tile([128, H], F32, tag="osb")
                        nc.vector.tensor_scalar_mul(o_sb[:], o_ps[:, 0:H], rc[:])
                        row = tci * 512 + k * 128
                        nc.sync.dma_start(out[row : row + 128, :], o_sb[:])

            # emission order: first half of projections; then phase-2 scores
            # over the ready key tiles (they overlap the second DMA half);
            # then the rest, pipelined.
            for sb in range(NTC):
                emit_proj_block(sb)
            emit_attn(0, 0, 16)
            for sb in range(NTC, NSB):
                emit_proj_block(sb)
            emit_attn(0, 16, NST)
            emit_attn(1, 0, 16)
            emit_epilogue(0)
            emit_attn(1, 16, NST)
            emit_epilogue(1)

    nc.compile()
    return nc


_NC_CACHE = None


def _get_module():
    global _NC_CACHE
    if _NC_CACHE is None:
        _NC_CACHE = _build_module()
    return _NC_CACHE


def _make_in_maps(x, Wq, Wk, Wv):
    xT = np.transpose(np.asarray(x, dtype=np.float32), (0, 2, 1))  # [B, C, T]
    # pre-tile for contiguous 256 KiB DMAs: [C,T] -> [NSB, NC_CH, 128, 512]
    wq = np.asarray(Wq, dtype=np.float32)
    wk = np.asarray(Wk, dtype=np.float32)
    wv = np.asarray(Wv, dtype=np.float32)
    # [Wk | 0]: scores contraction zero-padded to K=128 (f32r matmuls run at
    # half rate for K=64); [Wq | Wv]: the V^T rows double as finite padding
    # rows on the rhs side of the scores matmul.
    wkpad = np.ascontiguousarray(
        np.concatenate([wk, np.zeros_like(wk)], axis=1).reshape(NC_CH, 128, 2 * H)
    )
    wqv = np.ascontiguousarray(
        np.concatenate([wq, wv], axis=1).reshape(NC_CH, 128, 2 * H)
    )
    in_maps = []
    for core in range(N_CORES):
        b, h = divmod(core, 2)
        xt = xT[b]
        if h == 1:
            xt = np.concatenate([xt[:, TQ:], xt[:, :TQ]], axis=1)
        xt = np.ascontiguousarray(
            xt.reshape(NC_CH, 128, NSB, 512).transpose(2, 0, 1, 3)
        )
        in_maps.append({"xT": xt, "wkpad": wkpad, "wqv": wqv})
    return in_maps


def run(x, Wq, Wk, Wv, **spmd_kwargs):
    """Run on hardware; returns (output, BassKernelResults)."""
    nc = _get_module()
    in_maps = _make_in_maps(x, Wq, Wk, Wv)
    res = run_bass_kernel_spmd(nc, in_maps, core_ids=list(range(N_CORES)), **spmd_kwargs)
    out = np.empty((B, T, H), dtype=np.float32)
    for core in range(N_CORES):
        b, h = divmod(core, 2)
        out[b, h * TQ : (h + 1) * TQ, :] = res.results[core]["out"]
    return out, res


def kernel(x, Wq, Wk, Wv):
    out, _ = run(x, Wq, Wk, Wv)
    return out



# revision 11
# speedup vs baseline: 1.0715x; 1.0715x over previous
"""Single-head attention (B=4, T=4096, C=1024, H=64) on 8 trn2 NeuronCores.

Sharding: 8 shards = (batch b, query-half h).  Each core receives x[b]
pre-transposed to xT [C=1024, T=4096]; for h==1 the T columns are rotated by
2048 so that "this core's" 2048 queries are always columns 0:2048 (softmax is
permutation-invariant over keys).  SPMD: identical program on every core.

v2 (bf16 matmuls + two-engine exp):
  All matmul inputs are bf16: 1 cyc/row on the PE (f32r streams at 2) and
  half the HBM traffic for the x stream.  K^T and V^T come out of ONE
  projection pass ([Wk'|Wv] stationary, 128 rows) instead of two.

  The exp stage (8.4M elements -- would be ~70us on ScalarE alone) is
  split across two engines via the Schraudolph bit trick: scores reach
  PSUM already affine-mapped as s' = (16*log2(e))*s_qk + 16248.5 (the
  scale folded into Wk host-side; the offset added by two constant
  contraction rows K^T[64]=K^T[65]=1 against Q^T[64]=16192, Q^T[65]=56.5
  -- both exactly bf16-representable, and 16248.5 = 2^7*(127-sigma) is
  the offset that makes int16(s') reinterpreted as bf16 equal
  exp(0.125*s_qk)*(1+-3%)).  So the DVE computes its share of exp with a
  single f32->int16 convert, while ACT computes exact exp for its share
  (scale/bias un-map s').  attn@V consumes ex as bf16; the extra ones
  column of V makes the softmax denominator fall out of the same matmul.

Per-core phases:
  phase 1: stream xT bf16 ([128,2048] DMAs over 3 queues); PE: KV^T
           accumulated over 8 C-chunks ([128,512] out per block), Q^T for
           the query half; V^T tiles PE-transposed back to V [s,64] bf16.
  phase 2: per (query chunk 512) x (key pair-block 256): two score MMs
           [66,128]x[66,512] -> PSUM [128,1024]; exp halves (ACT exact /
           DVE Schraudolph) -> ex bf16 [128,2,512]; two MMs accumulate
           attn@V into PSUM [65,512] over 32 key tiles.  Epilogue:
           PE-transpose back to [t,65], multiply by the reciprocal of the
           exp-sum column, DMA out fp32.
"""

import os
import sys

for _p in ("/opt/trn_rl_repo", "/root/.axon_site/_ro/trn_rl_repo"):
    if os.path.isdir(_p) and _p not in sys.path:
        sys.path.append(_p)

import numpy as np
import ml_dtypes

import concourse.bacc as bacc
import concourse.mybir as mybir
import concourse.tile as tile
from concourse.bass_utils import run_bass_kernel_spmd
from concourse.masks import make_identity

B = 4
T = 4096
C = 1024
H = 64
TQ = T // 2  # queries per core
N_CORES = 8

F32 = mybir.dt.float32
BF16 = mybir.dt.bfloat16
I16 = mybir.dt.int16

NC_CH = C // 128  # 8 contraction chunks
NBLK = T // 512  # 8 key/source blocks of 512
NPAIR = T // 256  # 16 key pair-blocks of 256
NST = T // 128  # 32 key tiles
NTC = TQ // 512  # 4 query chunks of 512

# exp(s) ~= bits_as_bf16(trunc(23.083*s + 16248.5)); 23.083*... folded into Wk
K_FOLD = 0.125 * 128.0 / np.log(2.0)  # 23.0831
B_HI = 16192.0  # 2^7 * 126.5, exact bf16
B_LO = 56.5  # exact bf16; B_HI+B_LO = 16248.5 = 2^7*(127 - 0.0586)
ACT_SCALE = float(np.log(2.0) / 128.0)
ACT_BIAS = float(-(B_HI + B_LO) * np.log(2.0) / 128.0)

EXP = mybir.ActivationFunctionType.Exp
COPY = mybir.ActivationFunctionType.Copy


def _build_module():
    nc = bacc.Bacc("TRN2", target_bir_lowering=False, debug=False, num_devices=N_CORES)

    xT = nc.dram_tensor("xT", [NC_CH, 128, T], BF16, kind="ExternalInput").ap()
    wkv = nc.dram_tensor("wkv", [128, NC_CH, 128], BF16, kind="ExternalInput").ap()
    wq = nc.dram_tensor("wq", [128, NC_CH, H], BF16, kind="ExternalInput").ap()
    qbias = nc.dram_tensor("qbias", [2, TQ], BF16, kind="ExternalInput").ap()
    out = nc.dram_tensor("out", [TQ, H], F32, kind="ExternalOutput").ap()

    with tile.TileContext(nc) as tc:
        with (
            tc.tile_pool(name="const", bufs=1) as const_pool,
            tc.tile_pool(name="xt", bufs=16) as xt_pool,
            tc.tile_pool(name="big", bufs=1) as big_pool,
            tc.tile_pool(name="exp", bufs=6) as exp_pool,
            tc.tile_pool(name="outts", bufs=2) as outts_pool,
            tc.tile_pool(name="small", bufs=4) as small_pool,
            tc.tile_pool(name="p1", bufs=2, space="PSUM") as psum_p1,
            tc.tile_pool(name="psc", bufs=2, space="PSUM") as psum_sc,
            tc.tile_pool(name="pacc", bufs=2, space="PSUM") as psum_acc,
        ):
            # ---- constants ----
            wkv_sb = const_pool.tile([128, NC_CH, 128], BF16, tag="wkv")
            wq_sb = const_pool.tile([128, NC_CH, H], BF16, tag="wq")
            ident_bf = const_pool.tile([128, 128], BF16, tag="ident_bf")
            ident_f32 = const_pool.tile([65, 65], F32, tag="ident_f32")
            scratch = const_pool.tile([128, 1], F32, tag="scratch")
            act_bias = const_pool.tile([128, 1], F32, tag="act_bias")
            nc.gpsimd.memset(act_bias[:], ACT_BIAS)
            nc.sync.dma_start(wkv_sb[:], wkv)
            nc.sync.dma_start(wq_sb[:], wq)
            make_identity(nc, ident_bf[:])
            make_identity(nc, ident_f32[:])
            # pull the exp table load off the critical path
            nc.scalar.activation(scratch[:], ident_bf[:, 0:1], EXP)

            # ---- persistent activations ----
            # kt rows 0:64 = 23.083*K^T, rows 64:66 = ones (bias rows)
            kt_sb = big_pool.tile([66, T], BF16, tag="kt")
            vt_sb = big_pool.tile([128, T], BF16, tag="vt")  # rows 64:128 = V^T
            # qt rows 0:64 = Q^T, rows 64:66 = bias values
            qt_sb = big_pool.tile([66, TQ], BF16, tag="qt")
            va = big_pool.tile([128, NST, 65], BF16, tag="va")  # V | ones col
            nc.gpsimd.memset(kt_sb[64:66, :], 1.0)
            nc.sync.dma_start(qt_sb[64:66, :], qbias)
            nc.gpsimd.memset(va[:, :, 64:65], 1.0)

            # ---- x DMA (issued up front; queues rotate) ----
            dma_engines = (nc.sync, nc.gpsimd, nc.scalar)
            xts = {}
            for half in range(2):
                for c in range(NC_CH):
                    xt_t = xt_pool.tile([128, 2048], BF16, tag="xt")
                    dma_engines[(half * NC_CH + c) % 3].dma_start(
                        xt_t[:], xT[c, :, half * 2048 : (half + 1) * 2048]
                    )
                    xts[(half, c)] = xt_t

            # ---- phase 1: projections per 512-col block ----
            def emit_proj_block(sb):
                half, off = divmod(sb * 512, 2048)
                kv_ps = psum_p1.tile([128, 512], F32, tag="p1")
                for c in range(NC_CH):
                    nc.tensor.matmul(
                        kv_ps[:],
                        wkv_sb[:, c, :],
                        xts[(half, c)][:, off : off + 512],
                        start=(c == 0),
                        stop=(c == NC_CH - 1),
                    )
                blk = slice(sb * 512, (sb + 1) * 512)
                nc.vector.tensor_copy(kt_sb[0:64, blk], kv_ps[0:64, :])
                nc.scalar.activation(vt_sb[64:128, blk], kv_ps[64:128, :], COPY)
                if sb < NTC:
                    q_ps = psum_p1.tile([64, 512], F32, tag="p1")
                    for c in range(NC_CH):
                        nc.tensor.matmul(
                            q_ps[:],
                            wq_sb[:, c, :],
                            xts[(half, c)][:, off : off + 512],
                            start=(c == 0),
                            stop=(c == NC_CH - 1),
                        )
                    nc.vector.tensor_copy(qt_sb[0:64, blk], q_ps[:])
                # V^T -> V transposes
                for j in range(4):
                    st = sb * 4 + j
                    vt_ps = psum_p1.tile([128, 64], BF16, tag="p1")
                    nc.tensor.transpose(
                        vt_ps[:],
                        vt_sb[64:128, st * 128 : (st + 1) * 128],
                        ident_bf[64:128, 64:128],
                    )
                    nc.vector.tensor_copy(va[:, st, 0:64], vt_ps[:])

            # ---- phase 2: attention ----
            acc_tiles = {}

            def emit_attn(tcp, pair_lo, pair_hi):
                if tcp not in acc_tiles:
                    acc_tiles[tcp] = psum_acc.tile(
                        [65, 512], F32, tag="acc", name=f"av{tcp}"
                    )
                av_ps = acc_tiles[tcp]
                tq = slice(tcp * 512, (tcp + 1) * 512)
                pend = []  # delayed-by-one attn@V emission to keep PE streaming

                def flush():
                    for args, kwargs in pend:
                        nc.tensor.matmul(*args, **kwargs)
                    pend.clear()

                for pi in range(pair_lo, pair_hi):
                    sc_ps = psum_sc.tile([128, 1024], F32, tag="sc")
                    for k in range(2):
                        st = 2 * pi + k
                        nc.tensor.matmul(
                            sc_ps[:, k * 512 : (k + 1) * 512],
                            kt_sb[:, st * 128 : (st + 1) * 128],
                            qt_sb[:, tq],
                            start=True,
                            stop=True,
                        )
                    flush()
                    ex = exp_pool.tile([128, 2, 512], BF16, tag="exp")
                    for k in range(2):
                        sc_half = sc_ps[:, k * 512 : (k + 1) * 512]
                        if k == 0 or pi % 8 >= 6:
                            nc.scalar.activation(
                                ex[:, k, :], sc_half, EXP,
                                scale=ACT_SCALE, bias=act_bias[:],
                            )
                        else:
                            nc.vector.tensor_copy(ex[:, k, :].bitcast(I16), sc_half)
                    for k in range(2):
                        st = 2 * pi + k
                        pend.append(
                            (
                                (av_ps[:], va[:, st, :], ex[:, k, :]),
                                dict(start=(st == 0), stop=(st == NST - 1)),
                            )
                        )
                flush()

            def emit_epilogue(tcp):
                av_ps = acc_tiles[tcp]
                outt_sb = outts_pool.tile([65, 512], F32, tag="outts")
                nc.scalar.activation(outt_sb[:], av_ps[:], COPY)
                for j in range(4):
                    o_ps = psum_p1.tile([128, 65], F32, tag="p1")
                    nc.tensor.transpose(
                        o_ps[:], outt_sb[:, j * 128 : (j + 1) * 128], ident_f32[:]
                    )
                    rc = small_pool.tile([128, 1], F32, tag="rc")
                    nc.vector.reciprocal(rc[:], o_ps[:, 64:65])
                    o_sb = small_pool.tile([128, H], F32, tag="osb")
                    nc.vector.tensor_scalar_mul(o_sb[:], o_ps[:, 0:H], rc[:])
                    row = tcp * 512 + j * 128
                    dma_engines[j % 2].dma_start(out[row : row + 128, :], o_sb[:])

            # emission order: query-half projections; attention over ready
            # keys overlaps the second-half x DMA + projections.
            for sb in range(NTC):
                emit_proj_block(sb)
            emit_attn(0, 0, 8)
            for sb in range(NTC, NBLK):
                emit_proj_block(sb)
            emit_attn(0, 8, NPAIR)
            emit_epilogue(0)
            for tcp in range(1, NTC):
                emit_attn(tcp, 0, NPAIR)
                emit_epilogue(tcp)

    nc.compile()
    return nc


_NC_CACHE = None


def _get_module():
    global _NC_CACHE
    if _NC_CACHE is None:
        _NC_CACHE = _build_module()
    return _NC_CACHE


def _make_in_maps(x, Wq, Wk, Wv):
    x64 = np.asarray(x, dtype=np.float64)
    wq64 = np.asarray(Wq, dtype=np.float64)
    wkv64 = np.concatenate(
        [np.asarray(Wk, dtype=np.float64) * K_FOLD, np.asarray(Wv, dtype=np.float64)],
        axis=1,
    )  # [C, 128]
    wkv_t = np.ascontiguousarray(
        wkv64.reshape(NC_CH, 128, 128).transpose(1, 0, 2)
    ).astype(ml_dtypes.bfloat16)
    wq_t = np.ascontiguousarray(
        wq64.reshape(NC_CH, 128, H).transpose(1, 0, 2)
    ).astype(ml_dtypes.bfloat16)
    qbias_t = np.empty((2, TQ), dtype=ml_dtypes.bfloat16)
    qbias_t[0, :] = B_HI
    qbias_t[1, :] = B_LO
    in_maps = []
    for core in range(N_CORES):
        b, h = divmod(core, 2)
        xt = x64[b].T  # [C, T]
        if h == 1:
            xt = np.concatenate([xt[:, TQ:], xt[:, :TQ]], axis=1)
        xt = np.ascontiguousarray(xt.reshape(NC_CH, 128, T)).astype(ml_dtypes.bfloat16)
        in_maps.append({"xT": xt, "wkv": wkv_t, "wq": wq_t, "qbias": qbias_t})
    return in_maps


def run(x, Wq, Wk, Wv, **spmd_kwargs):
    """Run on hardware; returns (output, BassKernelResults)."""
    nc = _get_module()
    in_maps = _make_in_maps(x, Wq, Wk, Wv)
    res = run_bass_kernel_spmd(nc, in_maps, core_ids=list(range(N_CORES)), **spmd_kwargs)
    out = np.empty((B, T, H), dtype=np.float32)
    for core in range(N_CORES):
        b, h = divmod(core, 2)
        out[b, h * TQ : (h + 1) * TQ, :] = res.results[core]["out"]
    return out, res


def kernel(x, Wq, Wk, Wv):
    out, _ = run(x, Wq, Wk, Wv)
    return out
